# revision 10
# baseline (speedup 1.0000x reference)
"""Trainium2 Bass kernel for nn_Attention_85856396247857 — V3.

16-head causal attention with rotary embeddings, x:[2,2048,2048] fp32 in/out.
Sharding (8 cores): core c = (b, g): batch b = c // 4, head group g = c % 4
(tensor parallel; row-parallel output partials summed on the host).

V3 design notes (driven by TimelineSim profiling):
  - DMA consolidation: each input tensor loads with ONE dma_start (x and wq
    split once for pipelined starts).  Per-DMA fixed overhead is ~2.2us of
    serialized queue time; V2's 156 DMAs cost ~420us, V3 issues ~17.
  - All matmul operands bf16 (rel err ~3e-3 vs 2e-2 budget), x SBUF-resident.
  - A1 zero-stall: Q/K groups alternate on two 4-bank PSUM rings; the rotary
    permutation matmuls of group g are deferred into group g+1's matmul
    stream so the PE never waits on the DVE epilogue.
  - B: exp batched over pairs of S blocks (halves ACT's per-instruction
    PSUM-access overhead); softmax row sums via DVE block accumulation +
    one GPSIMD partition_all_reduce per (it,h) — the all-ones PE matmul of
    V1/V2 is gone.
  - C is interleaved into B per query tile so C's matmuls fill the PE while
    ACT works on the next tile's exps.  Output staged in SBUF, 4 DMAs.
"""

import os
import sys

import numpy as np

for _p in ("/opt/trn_rl_repo",):
    if _p not in sys.path and os.path.isdir(_p):
        sys.path.insert(0, _p)

import ml_dtypes  # noqa: E402

import concourse.bass as bass  # noqa: E402
import concourse.bass_isa as bass_isa  # noqa: E402
import concourse.mybir as mybir  # noqa: E402
import concourse.tile as tile  # noqa: E402
from concourse import bacc  # noqa: E402
from concourse.bass_utils import run_bass_kernel_spmd  # noqa: E402

F32 = mybir.dt.float32
F32R = mybir.dt.float32r
BF16 = mybir.dt.bfloat16
NP_BF16 = ml_dtypes.bfloat16

B, N, D = 2, 2048, 2048
H, DH = 16, 128
NCORES = 8
GROUPS = 4
HPC = H // GROUPS       # 4 heads per core
INNER_C = HPC * DH      # 512

KSL = D // 128          # 16 contraction slices
ST = 512                # A1 seq-tile width
NST = N // ST           # 4
NJB = N // 128          # 16 key blocks
NIT = N // 512          # 4 query tiles

# bf16 consts (cos/sin) and f32 consts (biases) layouts
COS_O, SIN_O = 0, N
CF_W = 2 * N
BQ_O, BK_O = 0, HPC
BVB_O = 2 * HPC
CB_W = BVB_O + INNER_C

_CACHE = {}
LAST_RESULTS = None


def _emit_a1(nc, tc, sx, phases, d, xt, wq_sb, wk_sb, cf_sb, cb_sb,
             perm_sb, qt_sb, kt_sb, qtmp_pool, tmp_pool, ps_q, ps_k):
    cos = cf_sb[:, COS_O:COS_O + N]
    sin = cf_sb[:, SIN_O:SIN_O + N]

    def emit_epilogue_pre(pqs, b_o):
        """ACT bias-copies into bf16 (frees the accumulation banks; the
        DVE stays clear for the rotary muls)."""
        qtmps = []
        for m in range(HPC):
            qtmp = qtmp_pool.tile([128, ST], BF16, tag="qtmp")
            nc.scalar.activation(qtmp[:], pqs[m][:],
                                 mybir.ActivationFunctionType.Identity,
                                 bias=cb_sb[:, b_o + m:b_o + m + 1])
            qtmps.append(qtmp)
        return qtmps

    def emit_epilogue_main(pool, tag, qtmps, dst, s0):
        """Rotary: bf16 perm matmuls (PE), ACT PSUM drains, 2x DVE muls."""
        psws = []
        for m in range(HPC):
            psw = pool.tile([128, ST], F32, tag=tag, name="psw")
            nc.tensor.matmul(psw[:], perm_sb[:], qtmps[m][:],
                             start=True, stop=True)
            psw_sb = tmp_pool.tile([128, ST], BF16, tag="t2")
            nc.scalar.activation(psw_sb[:], psw[:],
                                 mybir.ActivationFunctionType.Copy)
            psws.append(psw_sb)
        for m in range(HPC):
            t1 = tmp_pool.tile([128, ST], BF16, tag="t1")
            nc.vector.tensor_mul(t1[:], qtmps[m][:], cos[:, s0:s0 + ST])
            nc.vector.tensor_mul(psws[m][:], psws[m][:], sin[:, s0:s0 + ST])
            nc.vector.tensor_add(dst[:, m, s0:s0 + ST], t1[:], psws[m][:])

    pending = None
    for st in range(NST if "A1" in phases else 0):
        s0 = st * ST
        for w_sb, pool, tag, b_o, dst in (
                (wq_sb, ps_q, "ppq", BQ_O, qt_sb),
                (wk_sb, ps_k, "ppk", BK_O, kt_sb)):
            pqs = [pool.tile([128, ST], F32, tag=tag, name=f"p{tag}{_m}")
                   for _m in range(HPC)]
            for k in range(KSL):
                for m in range(HPC):
                    nc.tensor.matmul(
                        pqs[m][:], w_sb[:, k, m * 128:(m + 1) * 128],
                        xt[:, k, s0:s0 + ST],
                        start=(k == 0), stop=(k == KSL - 1))
                if k == 3 and pending is not None:
                    emit_epilogue_main(*pending)
                    pending = None
            qtmps = emit_epilogue_pre(pqs, b_o)
            pending = (pool, tag, qtmps, dst, s0)
    # the trailing epilogue is deferred into A2's first matmul group
    return (lambda: emit_epilogue_main(*pending)) if pending else None


def _emit_a2(nc, tc, sx, phases, d, xt, wv_sb, cb_sb, v_sb, ps_pool,
             epilogue_cb=None):
    for sb in range(NJB if "A2" in phases else 0):
        pv = ps_pool.tile([128, INNER_C], F32, tag="ppq", name="pv")
        for k in range(KSL):
            nc.tensor.matmul(
                pv[:], xt[:, k, sb * 128:(sb + 1) * 128], wv_sb[:, k, :],
                start=(k == 0), stop=(k == KSL - 1))
            if sb == 0 and k == 7 and epilogue_cb is not None:
                epilogue_cb()
        nc.vector.tensor_add(v_sb[:, sb, :], pv[:],
                             cb_sb[:, BVB_O:BVB_O + INNER_C])
    if "A2" not in phases and epilogue_cb is not None:
        epilogue_cb()


def _emit_b_tile(nc, tc, d, it, h, qt_sb, kt_sb, v_sb, ot_sb, bres,
                 filler=None, filler_steps=None):
    filler_steps = list(filler_steps or [])
    """Attention for one (query-tile, head): S^T blocks, paired exp,
    DVE-accumulated row sums + GPSIMD partition reduce, PV in PSUM."""
    (mask_sb, pt_pool, pacc_pool, rec_pool, ps_s, ps_o) = bres
    i0 = it * 512
    njb = 4 * it + 4
    qs = qt_sb[:, h, i0:i0 + 512]
    po_t = ps_o.tile([128, 512], F32, tag="po")
    pacc = pacc_pool.tile([128, 512], BF16, tag="pacc")

    def _flush(jb, off, p_ap):
        nc.tensor.matmul(
            po_t[:, off:],
            v_sb[:, jb, h * 128:(h + 1) * 128],
            p_ap[:, off:],
            start=(jb == 0), stop=(jb == njb - 1))

    pending = []
    pend_acc = []
    for pr in range(njb // 2):
        jb0, jb1 = 2 * pr, 2 * pr + 1
        dk0, dk1 = jb0 - 4 * it, jb1 - 4 * it
        off0 = dk0 * 128 if dk0 in (1, 2) else 0
        off1 = dk1 * 128 if dk1 in (1, 2) else 0
        ps_blk = ps_s.tile([128, 1024], F32, tag="ps")
        nc.tensor.matmul(
            ps_blk[:, off0:512],
            kt_sb[:, h, jb0 * 128:(jb0 + 1) * 128],
            qs[:, off0:], start=True, stop=True)
        nc.tensor.matmul(
            ps_blk[:, 512 + off1:1024],
            kt_sb[:, h, jb1 * 128:(jb1 + 1) * 128],
            qs[:, off1:], start=True, stop=True)
        p_sb = pt_pool.tile([128, 1024], BF16, tag="p")
        # one exp over both blocks; unwritten [0:off] PSUM cols produce
        # garbage in p that no consumer reads
        nc.scalar.activation(p_sb[:], ps_blk[:],
                             mybir.ActivationFunctionType.Exp)
        if filler_steps:
            filler_steps.pop(0)()
        for jb, off, half in ((jb0, off0, 0), (jb1, off1, 1)):
            p_ap = p_sb[:, half * 512:(half + 1) * 512]
            dk = jb - 4 * it
            if dk >= 0:
                nc.vector.tensor_mul(
                    p_ap[:, off:], p_ap[:, off:],
                    mask_sb[:, dk * 512 + off:(dk + 1) * 512])
            # row-sum accumulation on DVE
            if jb == 0:
                nc.vector.tensor_copy(pacc[:], p_ap[:])
            else:
                nc.vector.tensor_add(pacc[:, off:], pacc[:, off:],
                                     p_ap[:, off:])
            pending.append((jb, off, p_ap))
            if len(pending) > 3:
                _flush(*pending.pop(0))
    if filler is not None:
        filler()
    for step in filler_steps:
        step()
    for item in pending:
        _flush(*item)

    # broadcast row sums across partitions on the idle GPSIMD engine
    prsum = rec_pool.tile([128, 512], F32, tag="prs")
    nc.gpsimd.partition_all_reduce(prsum[:], pacc[:], 128,
                                   bass_isa.ReduceOp.add)
    rec = rec_pool.tile([128, 512], F32, tag="rec")
    nc.vector.reciprocal(rec[:], prsum[:])
    nc.vector.tensor_mul(ot_sb[:, h, i0:i0 + 512], po_t[:], rec[:])


def _emit_c_nt(nc, tc, d, so, nt, ot_sb, wo_sb, stage, ps_out):
    pout = ps_out.tile([128, 512], F32, tag="pout")
    for hh in range(HPC):
        nc.tensor.matmul(
            pout[:],
            ot_sb[:, hh, so * 128:(so + 1) * 128],
            wo_sb[:, hh, nt * 512:(nt + 1) * 512],
            start=(hh == 0), stop=(hh == HPC - 1))
    dst = stage[:, so % 4, nt * 512:(nt + 1) * 512]
    if nt % 2 == 0:
        nc.vector.tensor_copy(dst, pout[:])
    else:
        nc.scalar.activation(dst, pout[:],
                             mybir.ActivationFunctionType.Copy)


def _emit_c_block(nc, tc, d, so, ot_sb, wo_sb, stage, ps_out):
    for nt in range(D // 512):
        _emit_c_nt(nc, tc, d, so, nt, ot_sb, wo_sb, stage, ps_out)


def _build_program(phases=("A1", "A2", "B", "C"), repeat=1, hw_loop=0):
    import contextlib
    phases = set(phases)
    nc = bacc.Bacc("TRN2", target_bir_lowering=False, debug=False,
                   num_devices=NCORES)

    d = {}
    d["xtr"] = nc.dram_tensor("xtr", [D, N], BF16, kind="ExternalInput").ap()
    d["wq"] = nc.dram_tensor("wq", [D, INNER_C], BF16, kind="ExternalInput").ap()
    d["wk"] = nc.dram_tensor("wk", [D, INNER_C], BF16, kind="ExternalInput").ap()
    d["wv"] = nc.dram_tensor("wv", [D, INNER_C], BF16, kind="ExternalInput").ap()
    d["wo"] = nc.dram_tensor("wo", [INNER_C, D], BF16, kind="ExternalInput").ap()
    d["cf"] = nc.dram_tensor("cf", [128, CF_W], BF16, kind="ExternalInput").ap()
    d["cb"] = nc.dram_tensor("cb", [128, CB_W], F32, kind="ExternalInput").ap()
    d["perm"] = nc.dram_tensor("perm", [128, 128], BF16, kind="ExternalInput").ap()
    d["mask"] = nc.dram_tensor("mask", [128, 4 * 512], BF16, kind="ExternalInput").ap()
    out_d = nc.dram_tensor("out", [NIT, 128, 4, D], BF16,
                           kind="ExternalOutput").ap()

    with tile.TileContext(nc) as tc:
        with (tc.For_i(0, hw_loop) if hw_loop
              else contextlib.nullcontext()):
            _emit_body(nc, tc, phases, repeat, d, out_d)

    nc.compile()
    return nc


def _emit_body(nc, tc, phases, repeat, d, out_d):
    with tc.tile_pool(name="qkpool", bufs=1) as qk_pool:
        qt_sb = qk_pool.tile([128, HPC, N], BF16)
        kt_sb = qk_pool.tile([128, HPC, N], BF16)
        if "A1" not in phases:
            nc.gpsimd.memset(qt_sb[:], 0.0)
            nc.gpsimd.memset(kt_sb[:], 0.0)

        for rep in range(repeat):
            sx = f"_{rep}" if rep else ""
            with (
                tc.tile_pool(name="vpool" + sx, bufs=1) as v_pool,
                tc.tile_pool(name="cpool" + sx, bufs=1) as c_pool,
            ):
                v_sb = v_pool.tile([128, NJB, INNER_C], BF16)
                if "A2" not in phases:
                    nc.gpsimd.memset(v_sb[:], 0.0)
                cf_sb = c_pool.tile([128, CF_W], BF16)
                cb_sb = c_pool.tile([128, CB_W], F32)
                perm_sb = c_pool.tile([128, 128], BF16)
                mask_sb = c_pool.tile([128, 4 * 512], BF16)

                xtr_v = d["xtr"].rearrange("(ko p) n -> p ko n", p=128)
                wq_v = d["wq"].rearrange("(ko p) i -> p ko i", p=128)
                wk_v = d["wk"].rearrange("(ko p) i -> p ko i", p=128)
                wv_v = d["wv"].rearrange("(ko p) i -> p ko i", p=128)

                with (
                    tc.tile_pool(name="xpool" + sx, bufs=1) as x_pool,
                    tc.tile_pool(name="wpool" + sx, bufs=1) as w_pool,
                ):
                    xt = x_pool.tile([128, KSL, N], BF16)
                    wq_sb = w_pool.tile([128, KSL, INNER_C], BF16)
                    warm = w_pool.tile([128, 1], F32)
                    nc.scalar.activation(warm[:], warm[:],
                                         mybir.ActivationFunctionType.Exp)
                    wk_sb = w_pool.tile([128, KSL, INNER_C], BF16)
                    wv_sb = w_pool.tile([128, KSL, INNER_C], BF16)

                    # consolidated DMAs, ordered so arrival tracks the
                    # k-ordered consumption of the first A1 groups
                    nc.sync.dma_start(wq_sb[:, 0:4, :], wq_v[:, 0:4, :])
                    nc.sync.dma_start(xt[:, 0:4, 0:ST], xtr_v[:, 0:4, 0:ST])
                    nc.sync.dma_start(wq_sb[:, 4:, :], wq_v[:, 4:, :])
                    nc.sync.dma_start(xt[:, 4:, 0:ST], xtr_v[:, 4:, 0:ST])
                    nc.sync.dma_start(cb_sb[:], d["cb"][:])
                    nc.sync.dma_start(perm_sb[:], d["perm"][:])
                    nc.sync.dma_start(cf_sb[:], d["cf"][:])
                    nc.sync.dma_start(wk_sb[:], wk_v[:])
                    nc.sync.dma_start(xt[:, :, ST:2 * ST],
                                      xtr_v[:, :, ST:2 * ST])
                    nc.sync.dma_start(xt[:, :, 2 * ST:3 * ST],
                                      xtr_v[:, :, 2 * ST:3 * ST])
                    nc.sync.dma_start(wv_sb[:], wv_v[:])
                    nc.sync.dma_start(xt[:, :, 3 * ST:N],
                                      xtr_v[:, :, 3 * ST:N])
                    nc.sync.dma_start(mask_sb[:], d["mask"][:])

                    with (
                        tc.tile_pool(name="qtp" + sx, bufs=4) as qtmp_pool,
                        tc.tile_pool(name="tmp" + sx, bufs=2) as tmp_pool,
                        tc.tile_pool(name="ps_q" + sx, bufs=4,
                                     space="PSUM") as ps_q,
                        tc.tile_pool(name="ps_k" + sx, bufs=4,
                                     space="PSUM") as ps_k,
                    ):
                        a1_tail = _emit_a1(
                            nc, tc, sx, phases, d, xt, wq_sb, wk_sb,
                            cf_sb, cb_sb, perm_sb, qt_sb, kt_sb,
                            qtmp_pool, tmp_pool, ps_q, ps_k)

                        _emit_a2(nc, tc, sx, phases, d, xt, wv_sb,
                                 cb_sb, v_sb, ps_q, epilogue_cb=a1_tail)

                with (
                    tc.tile_pool(name="ot" + sx, bufs=1) as ot_pool,
                    tc.tile_pool(name="wopool" + sx, bufs=1) as wopool,
                    tc.tile_pool(name="stage" + sx, bufs=2) as stage_pool,
                    tc.tile_pool(name="ptile" + sx, bufs=4) as pt_pool,
                    tc.tile_pool(name="pacc" + sx, bufs=2) as pacc_pool,
                    tc.tile_pool(name="rec" + sx, bufs=2) as rec_pool,
                    tc.tile_pool(name="ps_s" + sx, bufs=2,
                                 space="PSUM") as ps_s,
                    tc.tile_pool(name="ps_o" + sx, bufs=2,
                                 space="PSUM") as ps_o,
                    tc.tile_pool(name="ps_out" + sx, bufs=2,
                                 space="PSUM") as ps_out,
                ):
                    ot_sb = ot_pool.tile([128, HPC, N], BF16)
                    if "B" not in phases:
                        nc.gpsimd.memset(ot_sb[:], 0.0)
                    wo_sb = wopool.tile([128, HPC, D], BF16)
                    nc.sync.dma_start(
                        wo_sb[:], d["wo"].rearrange("(hh p) dd -> p hh dd",
                                                    p=128))
                    bres = (mask_sb, pt_pool, pacc_pool, rec_pool,
                            ps_s, ps_o)

                    stage = [None]

                    def c_steps(itp, h):
                        if h == 0:
                            stage[0] = stage_pool.tile([128, 4, D], BF16,
                                                       tag="stg", name="stg")
                        steps = [
                            (lambda nt=nt: _emit_c_nt(
                                nc, tc, d, itp * 4 + h, nt, ot_sb, wo_sb,
                                stage[0], ps_out))
                            for nt in range(D // 512)]
                        if h == 1:
                            steps.append(lambda: nc.sync.dma_start(
                                out_d[itp, :, 0:2], stage[0][:, 0:2]))
                        if h == HPC - 1:
                            steps.append(lambda: nc.sync.dma_start(
                                out_d[itp, :, 2:4], stage[0][:, 2:4]))
                        return steps

                    def emit_c_block_staged(itp, h):
                        for s in c_steps(itp, h):
                            s()

                    if "B" in phases:
                        for it in range(NIT):
                            for h in range(HPC):
                                steps = None
                                if "C" in phases and it >= 1:
                                    steps = c_steps(it - 1, h)
                                _emit_b_tile(nc, tc, d, it, h, qt_sb, kt_sb,
                                             v_sb, ot_sb, bres,
                                             filler_steps=steps)
                    if "C" in phases or "B" not in phases:
                        for h in range(HPC):
                            emit_c_block_staged(NIT - 1, h)


def _host_consts():
    scale = DH ** -0.5
    inv_freq = 1.0 / (10000.0 ** (np.arange(0, DH, 2, dtype=np.float32) / DH))
    seq = np.arange(N, dtype=np.float32)
    freqs = np.einsum('i,j->ij', seq, inv_freq)
    pos = np.concatenate((freqs, freqs), axis=-1)
    cos_t = np.cos(pos).T.astype(np.float32)
    sin_t = np.sin(pos).T.astype(np.float32)
    sin_t[:64] *= -1.0

    perm = np.zeros((128, 128), dtype=np.float32)
    perm[(np.arange(128) + 64) % 128, np.arange(128)] = 1.0

    mask = np.zeros((128, 4 * 512), dtype=np.float32)
    jj = np.arange(128)[:, None]
    ii = np.arange(512)[None, :]
    for dk in range(4):
        mask[:, dk * 512:(dk + 1) * 512] = (jj + dk * 128 <= ii)
    return scale, cos_t, sin_t, perm, mask


def build_in_maps(inputs):
    return _build_in_maps(**inputs)


def _build_in_maps(x, Wq, bq, Wk, bk, Wv, bv, Wo, bo):
    x = np.ascontiguousarray(np.asarray(x, dtype=np.float32))
    Wq = np.asarray(Wq, dtype=np.float32)
    Wk = np.asarray(Wk, dtype=np.float32)
    Wv = np.asarray(Wv, dtype=np.float32)
    Wo = np.asarray(Wo, dtype=np.float32)
    bq = np.asarray(bq, dtype=np.float32)
    bk = np.asarray(bk, dtype=np.float32)
    bv = np.asarray(bv, dtype=np.float32)

    scale, cos_t, sin_t, perm, mask = _host_consts()

    in_maps = []
    for c in range(NCORES):
        b, g = c // GROUPS, c % GROUPS
        sl = slice(g * INNER_C, (g + 1) * INNER_C)
        cf = np.zeros((128, CF_W), dtype=np.float32)
        cf[:, COS_O:COS_O + N] = cos_t
        cf[:, SIN_O:SIN_O + N] = sin_t
        cb = np.zeros((128, CB_W), dtype=np.float32)
        cb[:, BQ_O:BQ_O + HPC] = (bq[sl] * scale).reshape(HPC, 128).T
        cb[:, BK_O:BK_O + HPC] = bk[sl].reshape(HPC, 128).T
        cb[:, BVB_O:BVB_O + INNER_C] = np.tile(bv[sl], (128, 1))
        in_maps.append({
            "xtr": np.ascontiguousarray(x[b].reshape(N, D).T).astype(NP_BF16),
            "wq": np.ascontiguousarray(Wq[:, sl] * scale).astype(NP_BF16),
            "wk": np.ascontiguousarray(Wk[:, sl]).astype(NP_BF16),
            "wv": np.ascontiguousarray(Wv[:, sl]).astype(NP_BF16),
            "wo": np.ascontiguousarray(Wo[sl, :]).astype(NP_BF16),
            "cf": cf.astype(NP_BF16),
            "cb": cb,
            "perm": perm.astype(NP_BF16),
            "mask": mask.astype(NP_BF16),
        })
    return in_maps


def assemble_out(arr):
    """[NIT, 128, 4, D] core output -> [N, D] float32."""
    return np.asarray(arr).astype(np.float32).transpose(0, 2, 1, 3).reshape(
        N, D)


def kernel(x, Wq, bq, Wk, bk, Wv, bv, Wo, bo):
    global LAST_RESULTS
    if "nc" not in _CACHE:
        _CACHE["nc"] = _build_program()
    nc = _CACHE["nc"]

    bo = np.asarray(bo, dtype=np.float32)
    in_maps = _build_in_maps(x, Wq, bq, Wk, bk, Wv, bv, Wo, bo)

    LAST_RESULTS = run_bass_kernel_spmd(nc, in_maps, core_ids=list(range(NCORES)))
    results = LAST_RESULTS.results

    out = np.zeros((B, N, D), dtype=np.float32)
    for c in range(NCORES):
        out[c // GROUPS] += assemble_out(results[c]["out"])
    out += bo
    return out
